# revision 3
# baseline (speedup 1.0000x reference)
"""Trainium2 Bass kernel for nn_Graphs (soft decision-graph probability propagation).

Reference math (G=4 graphs, B=128 batch, N=255 internal nodes, L=256 leaves,
F=512 features, J=8 jumps):
  b  = sigmoid(x @ W_g^T + bias_g)                  (per graph: B x N)
  M0 = softmax(M_left, axis=dest), M1 = softmax(M_right, axis=dest)
  q  = [b*(M1-M0)+M0 | leaf-identity]               (per (g,batch): 511x511)
  prob <- q @ prob, J times, starting from e0; return leaf probs.

Key restructure vs the f32r baseline:
  - everything on the PE runs in fp16 (1 cycle/row at any clock; f32r pays
    4 cycles/row when the moving free dim < 256, which all jump matmuls are).
    fp16 end-to-end rel err ~1e-3 (validated vs fp32 reference) against a
    2e-2 gate.
  - softmax denominators stay folded into c01 = [r0*(1-b), r1*b] (E matrices
    are raw exp, never normalized). To keep the fp16 scaled states out of
    subnormal range, the initial one-hot state carries a 2^15 scale, undone
    in the final PSUM->SBUF copy.
  - the leaf-block matrices are jump-invariant, so leaf projection runs ONCE
    at the end on the summed scaled states (sacc = sum_j upv_j, accumulated
    on the otherwise-idle GPSIMD engine), instead of 4 extra matmuls per jump.
  - b-path sigmoid uses reciprocal_approx_fast (~5x faster than DVE
    reciprocal; 18 bits is plenty ahead of fp16 rounding).
  - no PE warmup stream: nothing left in the kernel benefits enough from the
    full 2.4GHz p-state to justify 3us of dummy matmuls.

Sharding: 8 cores = (graph g = core//2) x (batch half h = core%2, 64 rows).
No cross-core communication. Host pre-transposes/pads inputs (fp16):
  - mm (128, 4*512): exp input blocks [el0|el1|er0|er1]; block t of el is
    M_left[g]^T rows t*128..t*128+127 (source nodes on partitions, dest on
    free; leaf dests shifted to cols 256..511, col 255 = -1e4 pad -> exp 0,
    source row 255 pad = 0 -> exp 1, harmless since state row 255 is 0).
  - wx (128, 4*320): feature k-tile blocks [W_g^T cols(256, pad col 255=0) |
    x_half^T cols(64)].
  - biasp (128, 2) f32: -bias_g per node half (ACT computes exp(-logit-bias)).
Output per core: (64, 256) batch-major leaf probs; host assembles to (B,L,G)
and applies the reference interval clamp.
"""

import numpy as np

G, B, N, L, F, J = 4, 128, 255, 256, 512, 8
BH = B // 2  # 64 batch rows per core
NCORES = 8
NEG = np.float32(-1e4)
SCALE = np.float32(2.0 ** 15)

_CACHE = {}


def _build_program():
    import concourse.mybir as mybir
    from concourse import bacc
    from concourse.tile import TileContext

    f32 = mybir.dt.float32
    f16 = mybir.dt.float16
    AF = mybir.ActivationFunctionType
    mult = mybir.AluOpType.mult
    add = mybir.AluOpType.add

    nc = bacc.Bacc(None)
    p_mm = nc.declare_dram_parameter("mm", [128, 4 * 512], f16, isOutput=False)
    p_wx = nc.declare_dram_parameter("wx", [128, 4 * 320], f16, isOutput=False)
    p_bias = nc.declare_dram_parameter("biasp", [128, 2], f32, isOutput=False)
    p_out = nc.declare_dram_parameter("out", [BH, 256], f32, isOutput=True)

    with TileContext(nc) as tc:
        with (
            tc.tile_pool(name="consts", bufs=1) as consts,
            tc.tile_pool(name="work", bufs=1) as work,
            tc.tile_pool(name="state", bufs=2) as state,
            tc.tile_pool(name="sacc", bufs=2) as saccp,
            tc.tile_pool(name="psum", bufs=2, space="PSUM") as psum,
            tc.tile_pool(name="psum_b", bufs=1, space="PSUM") as psum_b,
            tc.tile_pool(name="psum_leaf", bufs=1, space="PSUM") as psum_leaf,
        ):
            # ---- input DMAs ----
            # el blocks first on the sync queue (they head the ACT exp chain),
            # wx on the vector queue so the b-matmul path loads in parallel.
            mm = consts.tile([128, 4, 512], f16, tag="mm", name="mm")
            nc.sync.dma_start(mm[:, 0:2, :], p_mm[:, 0:1024])
            nc.sync.dma_start(mm[:, 2:4, :], p_mm[:, 1024:2048])
            wx = consts.tile([128, 4, 320], f16, tag="wx", name="wx")
            nc.scalar.dma_start(wx[:], p_wx[:])
            biasp = consts.tile([128, 2], f32, tag="biasp", name="biasp")
            nc.scalar.dma_start(biasp[:], p_bias[:])

            # ---- b-path matmuls: logits, node-major (2 node halves x 64) ----
            pb = psum_b.tile([128, 2, BH], f32, tag="pb", name="pb")
            for mh in range(2):
                for k in range(4):
                    nc.tensor.matmul(
                        pb[:, mh, :],
                        wx[:, k, mh * 128:(mh + 1) * 128],
                        wx[:, k, 256:320],
                        start=(k == 0), stop=(k == 3),
                    )

            # ---- E = exp(M^T) raw (fp16), row sums via ACT accumulator ----
            E = consts.tile([128, 4, 512], f16, tag="E", name="E")
            ssum = consts.tile([128, 4, 1], f32, tag="ssum", name="ssum")
            for blk in range(4):
                nc.scalar.activation(E[:, blk, :], mm[:, blk, :], AF.Exp,
                                     accum_out=ssum[:, blk, :])
            # eb = exp(-logit - bias) after the E exps so a late pb can't
            # stall them (ACT executes in order)
            eb = work.tile([128, 2, BH], f32, tag="eb", name="eb")
            for mh in range(2):
                nc.scalar.activation(eb[:, mh, :], pb[:, mh, :], AF.Exp,
                                     bias=biasp[:, mh:mh + 1], scale=-1.0)

            # ---- c01[t, 0] = r0*(1-b), c01[t, 1] = r1*b  (fp16) ----
            # sig = 1/(1+eb); 1-b = eb*sig; r = 1/rowsum from approx recip
            den = work.tile([128, 2, BH], f32, tag="den", name="den")
            nc.vector.tensor_scalar_add(den[:], eb[:], 1.0)
            sig = work.tile([128, 2, BH], f32, tag="sig", name="sig")
            nc.vector.reciprocal_approx_fast(sig[:], den[:])
            omb = work.tile([128, 2, BH], f32, tag="omb", name="omb")
            nc.vector.tensor_tensor(out=omb[:], in0=eb[:], in1=sig[:], op=mult)
            rec = consts.tile([128, 4, 1], f32, tag="rec", name="rec")
            nc.vector.reciprocal_approx_fast(rec[:], ssum[:])
            c01 = consts.tile([128, 2, 2, BH], f16, tag="c01", name="c01")
            nc.vector.tensor_tensor(
                out=c01[:, :, 0, :], in0=omb[:],
                in1=rec[:, 0:2, :].broadcast_to([128, 2, BH]), op=mult)
            nc.vector.tensor_tensor(
                out=c01[:, :, 1, :], in0=sig[:],
                in1=rec[:, 2:4, :].broadcast_to([128, 2, BH]), op=mult)

            # ---- initial state: one-hot root scaled by 2^15 ----
            z = consts.tile([128, 2, BH], f32, tag="z", name="z")
            nc.vector.memset(z[:], 0.0)
            nc.vector.memset(z[0:1, 0, :], float(SCALE))

            # ---- jump loop ----
            # upv[:, t, 0, :] = c0*u_t (E0 path), [:, t, 1, :] = c1*u_t (E1).
            # E block for (t, kind): blk = kind*2 + t.
            pq = None
            sacc_prev = None
            for j in range(J):
                upv = state.tile([128, 2, 2, BH], f16, tag="upv", name=f"upv{j}")
                s = z if j == 0 else pq
                nc.vector.tensor_tensor(
                    out=upv[:], in0=c01[:],
                    in1=s[:, :, None, :].broadcast_to([128, 2, 2, BH]), op=mult)
                # running state sum for the final leaf projection (GPSIMD)
                sacc = saccp.tile([128, 2, 2, BH], f16, tag="sacc", name=f"sacc{j}")
                if j == 0:
                    nc.gpsimd.tensor_copy(sacc[:], upv[:])
                else:
                    nc.gpsimd.tensor_tensor(out=sacc[:], in0=sacc_prev[:],
                                            in1=upv[:], op=add)
                sacc_prev = sacc
                if j < J - 1:
                    pq = psum.tile([128, 2, BH], f32, tag="pq", name=f"pq{j}")
                    for mt in range(2):
                        ms = slice(mt * 128, (mt + 1) * 128)
                        for i, (t, kind) in enumerate(
                                ((0, 0), (1, 0), (0, 1), (1, 1))):
                            nc.tensor.matmul(
                                pq[:, mt, :], E[:, kind * 2 + t, ms],
                                upv[:, t, kind, :],
                                start=(i == 0), stop=(i == 3),
                            )

            # ---- leaf projection, once: batch-major (64, 256) ----
            pleaf = psum_leaf.tile([BH, 256], f32, tag="pl", name="pl")
            for i, (t, kind) in enumerate(((0, 0), (1, 0), (0, 1), (1, 1))):
                nc.tensor.matmul(
                    pleaf[:], sacc_prev[:, t, kind, :],
                    E[:, kind * 2 + t, 256:512],
                    start=(i == 0), stop=(i == 3),
                )

            # ---- output: undo the 2^15 state scale in the copy ----
            o = work.tile([BH, 256], f32, tag="o", name="o")
            nc.vector.tensor_scalar_mul(o[:], pleaf[:], float(1.0 / SCALE))
            nc.sync.dma_start(p_out[:, :], o[:])

    nc.finalize()
    return nc


def _get_program():
    if "nc" not in _CACHE:
        _CACHE["nc"] = _build_program()
    return _CACHE["nc"]


def _prep_inputs(x, W, bias, M_left, M_right):
    """Host-side shard + layout prep. Core c -> graph c//2, batch half c%2."""
    in_maps = []
    mm_g, wt_g, bias_g = [], [], []
    for g in range(G):
        mm = np.zeros((4, 128, 512), np.float32)
        for side, M in ((0, M_left), (1, M_right)):
            mt = np.zeros((256, 512), np.float32)
            t = M[g].T  # (255, 511)
            mt[0:255, 0:255] = t[:, 0:255]
            mt[0:255, 256:512] = t[:, 255:511]
            mt[0:255, 255] = NEG
            mm[side * 2 + 0] = mt[0:128]
            mm[side * 2 + 1] = mt[128:256]
        mm_g.append(np.ascontiguousarray(
            mm.transpose(1, 0, 2).reshape(128, 2048)).astype(np.float16))
        wt = np.zeros((512, 256), np.float32)
        wt[:, 0:255] = W[g].T
        wt_g.append(wt)
        bp = np.zeros((128, 2), np.float32)
        bp[0:127, 1] = -bias[g][128:255]
        bp[:, 0] = -bias[g][0:128]
        bias_g.append(bp)
    xt_h = [np.ascontiguousarray(x[h * BH:(h + 1) * BH].T) for h in range(2)]
    wx_gh = {}
    for g in range(G):
        for h in range(2):
            wxf = np.zeros((128, 4, 320), np.float32)
            for k in range(4):
                wxf[:, k, 0:256] = wt_g[g][k * 128:(k + 1) * 128]
                wxf[:, k, 256:320] = xt_h[h][k * 128:(k + 1) * 128]
            wx_gh[(g, h)] = np.ascontiguousarray(
                wxf.reshape(128, 1280)).astype(np.float16)
    for c in range(NCORES):
        g, h = c // 2, c % 2
        in_maps.append({
            "mm": mm_g[g], "wx": wx_gh[(g, h)], "biasp": bias_g[g],
        })
    return in_maps


def _assemble(results):
    eps = np.float32(1e-5)
    ret = np.empty((B, L, G), np.float32)
    for c in range(NCORES):
        g, h = c // 2, c % 2
        ret[h * BH:(h + 1) * BH, :, g] = results[c]["out"]
    ret = np.where(ret > 0.0, ret, eps)
    ret = np.where(ret < 1.0, ret, np.float32(1.0) - eps)
    return ret.astype(np.float32)


def run_on_device(in_maps, trace=False, **kw):
    from concourse.bass_utils import run_bass_kernel_spmd
    nc = _get_program()
    return run_bass_kernel_spmd(nc, in_maps, list(range(NCORES)), trace=trace, **kw)


def kernel(x, W, bias, M_left, M_right):
    in_maps = _prep_inputs(
        np.asarray(x, np.float32), np.asarray(W, np.float32),
        np.asarray(bias, np.float32), np.asarray(M_left, np.float32),
        np.asarray(M_right, np.float32),
    )
    res = run_on_device(in_maps)
    return _assemble(res.results)


# revision 9
# speedup vs baseline: 1.0270x; 1.0270x over previous
"""Trainium2 Bass kernel for nn_Graphs (soft decision-graph probability propagation).

Reference math (G=4 graphs, B=128 batch, N=255 internal nodes, L=256 leaves,
F=512 features, J=8 jumps):
  b  = sigmoid(x @ W_g^T + bias_g)                  (per graph: B x N)
  M0 = softmax(M_left, axis=dest), M1 = softmax(M_right, axis=dest)
  q  = [b*(M1-M0)+M0 | leaf-identity]               (per (g,batch): 511x511)
  prob <- q @ prob, J times, starting from e0; return leaf probs.

Structure (fp16 everywhere on the PE; end-to-end rel err ~1e-3 vs a 2e-2
gate, validated against the fp32 reference in numpy):
  - softmax denominators folded into c01 = [r0*(1-b), r1*b]; E matrices are
    raw exp(M^T), never normalized. The initial one-hot state carries a 2^15
    scale (undone in the output copy) to keep fp16 scaled states normal.
  - leaf projection hoisted out of the jump loop: leaf mass = E_leaf^T @
    (sum_j state_j). The running sum accumulates on the GPSIMD engine; on the
    last jump the projection splits into sacc_6 part (runs during the last
    DVE op) + upv_7 part.
  - per jump, the DVE state-scaling op is split by source half so the PE can
    start its first 4 matmuls one DVE op earlier.
  - DMA: three queues in parallel (sync: el, scalar: er, gpsimd: wx), bias
    folded into the wx tile as fp16 columns to avoid a tiny 128-descriptor
    DMA; output DMA split across the two HWDGE queues.
  - b-path sigmoid via exp (no ACT table swap) + reciprocal_approx_fast; the
    two small eb exps issue after the four big E exps (ACT runs in order and
    pb can be late without stalling them).

Sharding: 8 cores = (graph g = core//2) x (batch half h = core%2, 64 rows).
Host prep (fp16): mm (128, 4*512) exp-input blocks [el0|el1|er0|er1], block t
of e* is M_*[g]^T rows t*128..t*128+127 (source nodes on partitions, dest on
free; leaf dests in cols 256..511, col 255 = -1e4 pad -> exp 0, source row
255 pad = 0 -> exp 1, harmless because state row 255 stays 0). wx (128,
4*320+2): feature k-tile blocks [W_g^T (256 cols, pad col 255=0) | x_half^T
(64 cols)] + 2 fp16 cols of -bias_g per node half.
Output per core: (64, 256) f32 batch-major leaf probs; host assembles to
(B,L,G) and applies the reference interval clamp.
"""

import numpy as np

G, B, N, L, F, J = 4, 128, 255, 256, 512, 8
BH = B // 2  # 64 batch rows per core
NCORES = 8
NEG = np.float32(-1e4)
SCALE = np.float32(2.0 ** 15)

_CACHE = {}


def _build_program():
    import concourse.mybir as mybir
    from concourse import bacc
    from concourse.tile import TileContext

    f32 = mybir.dt.float32
    f16 = mybir.dt.float16
    AF = mybir.ActivationFunctionType
    mult = mybir.AluOpType.mult
    add = mybir.AluOpType.add

    nc = bacc.Bacc(None)
    p_mm = nc.declare_dram_parameter("mm", [128, 4 * 512], f16, isOutput=False)
    p_wx = nc.declare_dram_parameter("wx", [128, 4 * 320 + 2], f16, isOutput=False)
    p_out = nc.declare_dram_parameter("out", [BH, 256], f32, isOutput=True)

    with TileContext(nc) as tc:
        with (
            tc.tile_pool(name="consts", bufs=1) as consts,
            tc.tile_pool(name="work", bufs=1) as work,
            tc.tile_pool(name="state", bufs=2) as state,
            tc.tile_pool(name="sacc", bufs=2) as saccp,
            tc.tile_pool(name="psum", bufs=2, space="PSUM") as psum,
            tc.tile_pool(name="psum_b", bufs=1, space="PSUM") as psum_b,
            tc.tile_pool(name="psum_leaf", bufs=1, space="PSUM") as psum_leaf,
        ):
            # ---- input DMAs, one per queue so dispatch runs in parallel ----
            mm = consts.tile([128, 4, 512], f16, tag="mm", name="mm")
            nc.sync.dma_start(mm[:, 0:2, :], p_mm[:, 0:1024])
            nc.scalar.dma_start(mm[:, 2:4, :], p_mm[:, 1024:2048])
            wx = consts.tile([128, 1282], f16, tag="wx", name="wx")
            nc.gpsimd.dma_start(wx[:], p_wx[:])

            # ---- b-path matmuls: logits, node-major (2 node halves x 64) ----
            pb = psum_b.tile([128, 2, BH], f32, tag="pb", name="pb")
            for mh in range(2):
                for k in range(4):
                    o = k * 320
                    nc.tensor.matmul(
                        pb[:, mh, :],
                        wx[:, o + mh * 128:o + (mh + 1) * 128],
                        wx[:, o + 256:o + 320],
                        start=(k == 0), stop=(k == 3),
                    )
            # bias rides in wx as fp16; ACT bias APs must be f32
            biasf = consts.tile([128, 2], f32, tag="biasf", name="biasf")
            nc.vector.tensor_copy(biasf[:], wx[:, 1280:1282])

            # ---- E = exp(M^T) raw (fp16), row sums via ACT accumulator ----
            E = consts.tile([128, 4, 512], f16, tag="E", name="E")
            ssum = consts.tile([128, 4, 1], f32, tag="ssum", name="ssum")
            for blk in range(4):
                nc.scalar.activation(E[:, blk, :], mm[:, blk, :], AF.Exp,
                                     accum_out=ssum[:, blk, :])
            eb = work.tile([128, 2, BH], f32, tag="eb", name="eb")
            for mh in range(2):
                nc.scalar.activation(eb[:, mh, :], pb[:, mh, :], AF.Exp,
                                     bias=biasf[:, mh:mh + 1], scale=-1.0)

            # ---- c01[t, 0] = r0*(1-b), c01[t, 1] = r1*b  (fp16) ----
            rec = consts.tile([128, 4, 1], f32, tag="rec", name="rec")
            nc.vector.reciprocal_approx_fast(rec[:], ssum[:])
            den = work.tile([128, 2, BH], f32, tag="den", name="den")
            nc.vector.tensor_scalar_add(den[:], eb[:], 1.0)
            sig = work.tile([128, 2, BH], f32, tag="sig", name="sig")
            nc.vector.reciprocal_approx_fast(sig[:], den[:])
            omb = work.tile([128, 2, BH], f32, tag="omb", name="omb")
            nc.vector.tensor_tensor(out=omb[:], in0=eb[:], in1=sig[:], op=mult)
            c01 = consts.tile([128, 2, 2, BH], f16, tag="c01", name="c01")
            nc.vector.tensor_tensor(
                out=c01[:, :, 0, :], in0=omb[:],
                in1=rec[:, 0:2, :].broadcast_to([128, 2, BH]), op=mult)
            nc.vector.tensor_tensor(
                out=c01[:, :, 1, :], in0=sig[:],
                in1=rec[:, 2:4, :].broadcast_to([128, 2, BH]), op=mult)

            # ---- initial state: one-hot root scaled by 2^15 ----
            z = consts.tile([128, 2, BH], f32, tag="z", name="z")
            nc.vector.memset(z[:], 0.0)
            nc.vector.memset(z[0:1, 0, :], float(SCALE))

            # ---- jump loop, software-pipelined ----
            # upv[:, t, 0, :] = c0*u_t (E0 path), [:, t, 1, :] = c1*u_t (E1).
            # E block for (t, kind): blk = kind*2 + t.
            # Matmuls group by DEST half mt into separate PSUM tiles; dest
            # half mt feeds source half t=mt next jump, so the DVE op for
            # upv[t=0] of jump j+1 runs as soon as jump j's mt=0 group (4
            # matmuls) stops, overlapping the mt=1 group -- the PE stream
            # stays gapless and the DVE ops hide under it.
            sacc_prev = saccp.tile([128, 2, 2, BH], f16, tag="sacc", name="sacc_init")
            nc.gpsimd.memset(sacc_prev[:], 0.0)
            pq = None
            upv = None
            for j in range(J):
                upv = state.tile([128, 2, 2, BH], f16, tag="upv", name=f"upv{j}")
                for t in range(2):
                    s = z[:, t, None, :] if j == 0 else pq[t][:, None, :]
                    nc.vector.tensor_tensor(
                        out=upv[:, t, :, :], in0=c01[:, t, :, :],
                        in1=s.broadcast_to([128, 2, BH]), op=mult)
                if j < J - 1:
                    pq = [psum.tile([128, BH], f32, tag=f"pq{mt}", name=f"pq{mt}_{j}")
                          for mt in range(2)]
                    for mt in range(2):
                        ms = slice(mt * 128, (mt + 1) * 128)
                        for i, (t, kind) in enumerate(
                                ((0, 0), (0, 1), (1, 0), (1, 1))):
                            nc.tensor.matmul(
                                pq[mt][:], E[:, kind * 2 + t, ms],
                                upv[:, t, kind, :],
                                start=(i == 0), stop=(i == 3),
                            )
                # running state sum for the final leaf projection (GPSIMD);
                # the last jump skips it (its leaf part projects directly)
                if j < J - 1:
                    sacc = saccp.tile([128, 2, 2, BH], f16, tag="sacc", name=f"sacc{j}")
                    nc.gpsimd.tensor_tensor(out=sacc[:], in0=sacc_prev[:],
                                            in1=upv[:], op=add)
                    sacc_prev = sacc

            # ---- leaf projection: sacc_6 part (overlaps the last DVE op),
            # then the upv_7 part ----
            pleaf = psum_leaf.tile([BH, 256], f32, tag="pl", name="pl")
            for i, (t, kind) in enumerate(((0, 0), (1, 0), (0, 1), (1, 1))):
                nc.tensor.matmul(
                    pleaf[:], sacc_prev[:, t, kind, :],
                    E[:, kind * 2 + t, 256:512],
                    start=(i == 0), stop=False,
                )
            for i, (t, kind) in enumerate(((0, 0), (1, 0), (0, 1), (1, 1))):
                nc.tensor.matmul(
                    pleaf[:], upv[:, t, kind, :],
                    E[:, kind * 2 + t, 256:512],
                    start=False, stop=(i == 3),
                )

            # ---- output: undo the 2^15 state scale in the copy; DMA split
            # across both HWDGE queues for parallel descriptor dispatch ----
            o = work.tile([BH, 256], f32, tag="o", name="o")
            nc.vector.tensor_scalar_mul(o[:], pleaf[:], float(1.0 / SCALE))
            nc.sync.dma_start(p_out[:, 0:128], o[:, 0:128])
            nc.scalar.dma_start(p_out[:, 128:256], o[:, 128:256])

    nc.finalize()
    return nc


def _get_program():
    if "nc" not in _CACHE:
        _CACHE["nc"] = _build_program()
    return _CACHE["nc"]


def _prep_inputs(x, W, bias, M_left, M_right):
    """Host-side shard + layout prep. Core c -> graph c//2, batch half c%2."""
    in_maps = []
    mm_g, wt_g, bias_g = [], [], []
    for g in range(G):
        mm = np.zeros((4, 128, 512), np.float32)
        for side, M in ((0, M_left), (1, M_right)):
            mt = np.zeros((256, 512), np.float32)
            t = M[g].T  # (255, 511)
            mt[0:255, 0:255] = t[:, 0:255]
            mt[0:255, 256:512] = t[:, 255:511]
            mt[0:255, 255] = NEG
            mm[side * 2 + 0] = mt[0:128]
            mm[side * 2 + 1] = mt[128:256]
        mm_g.append(np.ascontiguousarray(
            mm.transpose(1, 0, 2).reshape(128, 2048)).astype(np.float16))
        wt = np.zeros((512, 256), np.float32)
        wt[:, 0:255] = W[g].T
        wt_g.append(wt)
        bp = np.zeros((128, 2), np.float32)
        bp[:, 0] = -bias[g][0:128]
        bp[0:127, 1] = -bias[g][128:255]
        bias_g.append(bp)
    xt_h = [np.ascontiguousarray(x[h * BH:(h + 1) * BH].T) for h in range(2)]
    wx_gh = {}
    for g in range(G):
        for h in range(2):
            wxf = np.zeros((128, 1282), np.float32)
            for k in range(4):
                wxf[:, k * 320:k * 320 + 256] = wt_g[g][k * 128:(k + 1) * 128]
                wxf[:, k * 320 + 256:k * 320 + 320] = xt_h[h][k * 128:(k + 1) * 128]
            wxf[:, 1280:1282] = bias_g[g]
            wx_gh[(g, h)] = np.ascontiguousarray(wxf).astype(np.float16)
    for c in range(NCORES):
        g, h = c // 2, c % 2
        in_maps.append({"mm": mm_g[g], "wx": wx_gh[(g, h)]})
    return in_maps


def _assemble(results):
    eps = np.float32(1e-5)
    ret = np.empty((B, L, G), np.float32)
    for c in range(NCORES):
        g, h = c // 2, c % 2
        ret[h * BH:(h + 1) * BH, :, g] = results[c]["out"]
    ret = np.where(ret > 0.0, ret, eps)
    ret = np.where(ret < 1.0, ret, np.float32(1.0) - eps)
    return ret.astype(np.float32)


def run_on_device(in_maps, trace=False, **kw):
    from concourse.bass_utils import run_bass_kernel_spmd
    nc = _get_program()
    return run_bass_kernel_spmd(nc, in_maps, list(range(NCORES)), trace=trace, **kw)


def kernel(x, W, bias, M_left, M_right):
    in_maps = _prep_inputs(
        np.asarray(x, np.float32), np.asarray(W, np.float32),
        np.asarray(bias, np.float32), np.asarray(M_left, np.float32),
        np.asarray(M_right, np.float32),
    )
    res = run_on_device(in_maps)
    return _assemble(res.results)


# revision 14
# speedup vs baseline: 1.0790x; 1.0506x over previous
"""Trainium2 Bass kernel for nn_Graphs (soft decision-graph probability propagation).

Reference math (G=4 graphs, B=128 batch, N=255 internal nodes, L=256 leaves,
F=512 features, J=8 jumps):
  b  = sigmoid(x @ W_g^T + bias_g)                  (per graph: B x N)
  M0 = softmax(M_left, axis=dest), M1 = softmax(M_right, axis=dest)
  q  = [b*(M1-M0)+M0 | leaf-identity]               (per (g,batch): 511x511)
  prob <- q @ prob, J times, starting from e0; return leaf probs.

Structure (fp16 everywhere on the PE; end-to-end rel err ~1e-3 vs a 2e-2
gate, validated against the fp32 reference in numpy):
  - softmax denominators folded into c01 = [r0*(1-b), r1*b]; E matrices are
    raw exp(M^T), never normalized. The initial one-hot state carries a 2^15
    scale (undone in the output copy) to keep fp16 scaled states normal.
  - leaf projection hoisted out of the jump loop: leaf mass = E_leaf^T @
    (sum_j state_j). The running sum accumulates on the GPSIMD engine; on the
    last jump the projection splits into sacc_6 part (runs during the last
    DVE op) + upv_7 part.
  - per jump, the DVE state-scaling op is split by source half so the PE can
    start its first 4 matmuls one DVE op earlier.
  - DMA: three queues in parallel (sync: el, scalar: er, gpsimd: wx), bias
    folded into the wx tile as fp16 columns to avoid a tiny 128-descriptor
    DMA; output DMA split across the two HWDGE queues.
  - b-path sigmoid via exp (no ACT table swap) + reciprocal_approx_fast; the
    two small eb exps issue after the four big E exps (ACT runs in order and
    pb can be late without stalling them).

Sharding: 8 cores = (graph g = core//2) x (batch half h = core%2, 64 rows).
Host prep (fp16): mm (128, 4*512) exp-input blocks [el0|el1|er0|er1], block t
of e* is M_*[g]^T rows t*128..t*128+127 (source nodes on partitions, dest on
free; leaf dests in cols 256..511, col 255 = -1e4 pad -> exp 0, source row
255 pad = 0 -> exp 1, harmless because state row 255 stays 0). wx (128,
4*320+2): feature k-tile blocks [W_g^T (256 cols, pad col 255=0) | x_half^T
(64 cols)] + 2 fp16 cols of -bias_g per node half.
Output per core: (64, 256) f32 batch-major leaf probs; host assembles to
(B,L,G) and applies the reference interval clamp.
"""

import numpy as np

G, B, N, L, F, J = 4, 128, 255, 256, 512, 8
BH = B // 2  # 64 batch rows per core
NCORES = 8
NEG = np.float32(-1e4)
SCALE = np.float32(2.0 ** 15)

_CACHE = {}


def _build_program():
    import concourse.mybir as mybir
    from concourse import bacc
    from concourse.tile import TileContext

    f32 = mybir.dt.float32
    f16 = mybir.dt.float16
    AF = mybir.ActivationFunctionType
    mult = mybir.AluOpType.mult
    add = mybir.AluOpType.add

    nc = bacc.Bacc(None)
    p_mm = nc.declare_dram_parameter("mm", [128, 4 * 512], f16, isOutput=False)
    p_wx = nc.declare_dram_parameter("wx", [128, 4 * 320 + 2], f16, isOutput=False)
    p_out = nc.declare_dram_parameter("out", [BH, 256], f32, isOutput=True)

    with TileContext(nc) as tc:
        with (
            tc.tile_pool(name="consts", bufs=1) as consts,
            tc.tile_pool(name="work", bufs=1) as work,
            tc.tile_pool(name="state", bufs=2) as state,
            tc.tile_pool(name="sacc", bufs=2) as saccp,
            tc.tile_pool(name="psum", bufs=2, space="PSUM") as psum,
            tc.tile_pool(name="psum_b", bufs=1, space="PSUM") as psum_b,
            tc.tile_pool(name="psum_leaf", bufs=1, space="PSUM") as psum_leaf,
            tc.tile_pool(name="psum_w", bufs=2, space="PSUM") as psum_w,
        ):
            # ---- input DMAs: el halves head the sync queue (they gate the
            # ACT exp chain); wx heads the scalar queue (it gates the b-path
            # logits), er behind it still lands before the ACT chain needs it.
            mm = consts.tile([128, 4, 512], f16, tag="mm", name="mm")
            nc.sync.dma_start(mm[:, 0, :], p_mm[:, 0:512])
            nc.sync.dma_start(mm[:, 1, :], p_mm[:, 512:1024])
            wx = consts.tile([128, 1282], f16, tag="wx", name="wx")
            nc.scalar.dma_start(wx[:], p_wx[:])
            nc.scalar.dma_start(mm[:, 2:4, :], p_mm[:, 1024:2048])

            # ---- b-path matmuls: logits, node-major (2 node halves x 64) ----
            pb = psum_b.tile([128, 2, BH], f32, tag="pb", name="pb")
            for mh in range(2):
                for k in range(4):
                    o = k * 320
                    nc.tensor.matmul(
                        pb[:, mh, :],
                        wx[:, o + mh * 128:o + (mh + 1) * 128],
                        wx[:, o + 256:o + 320],
                        start=(k == 0), stop=(k == 3),
                    )
            # bias rides in wx as fp16; ACT bias APs must be f32
            biasf = consts.tile([128, 2], f32, tag="biasf", name="biasf")
            nc.vector.tensor_copy(biasf[:], wx[:, 1280:1282])

            # ---- E = exp(M^T) raw (fp16), row sums via ACT accumulator ----
            # ACT order: el exps, then the two small eb exps (pb is ready by
            # then thanks to wx heading the scalar queue), then er exps -- so
            # the b-path DVE chain runs concurrently with the er exps.
            E = consts.tile([128, 4, 512], f16, tag="E", name="E")
            ssum = consts.tile([128, 4, 1], f32, tag="ssum", name="ssum")
            for blk in range(2):
                nc.scalar.activation(E[:, blk, :], mm[:, blk, :], AF.Exp,
                                     accum_out=ssum[:, blk, :])
            eb = work.tile([128, 2, BH], f32, tag="eb", name="eb")
            for mh in range(2):
                nc.scalar.activation(eb[:, mh, :], pb[:, mh, :], AF.Exp,
                                     bias=biasf[:, mh:mh + 1], scale=-1.0)
            for blk in range(2, 4):
                nc.scalar.activation(E[:, blk, :], mm[:, blk, :], AF.Exp,
                                     accum_out=ssum[:, blk, :])

            # ---- c01[t, 0] = r0*(1-b), c01[t, 1] = r1*b  (fp16) ----
            # rec split so c0 completes while the er exps still run
            rec = consts.tile([128, 4, 1], f32, tag="rec", name="rec")
            nc.vector.reciprocal_approx_fast(rec[:, 0:2, :], ssum[:, 0:2, :])
            den = work.tile([128, 2, BH], f32, tag="den", name="den")
            nc.vector.tensor_scalar_add(den[:], eb[:], 1.0)
            sig = work.tile([128, 2, BH], f32, tag="sig", name="sig")
            nc.vector.reciprocal_approx_fast(sig[:], den[:])
            omb = work.tile([128, 2, BH], f32, tag="omb", name="omb")
            nc.vector.tensor_tensor(out=omb[:], in0=eb[:], in1=sig[:], op=mult)
            c01 = consts.tile([128, 2, 2, BH], f16, tag="c01", name="c01")
            nc.vector.tensor_tensor(
                out=c01[:, :, 0, :], in0=omb[:],
                in1=rec[:, 0:2, :].broadcast_to([128, 2, BH]), op=mult)
            nc.vector.reciprocal_approx_fast(rec[:, 2:4, :], ssum[:, 2:4, :])
            nc.vector.tensor_tensor(
                out=c01[:, :, 1, :], in0=sig[:],
                in1=rec[:, 2:4, :].broadcast_to([128, 2, BH]), op=mult)

            # ---- initial state: one-hot root scaled by 2^15 ----
            z = consts.tile([128, 2, BH], f32, tag="z", name="z")
            nc.vector.memset(z[:], 0.0)
            nc.vector.memset(z[0:1, 0, :], float(SCALE))
            # tiny moving tile for PE keep-warm dummy matmuls
            dwarm = consts.tile([128, 4], f16, tag="dwarm", name="dwarm")
            nc.vector.memset(dwarm[:], 0.0)

            def warm(n, nm):
                # dummy matmuls with no upstream deps: they fill the PE's
                # inter-jump idle gap so the p-state clock never drops
                # (a cold first matmul costs ~4x). One accumulation group
                # into a throwaway PSUM tile nobody reads.
                pw = psum_w.tile([128, 4], f32, tag="pw", name=nm)
                for i in range(n):
                    nc.tensor.matmul(pw[:], E[:, 0, 0:128], dwarm[:],
                                     start=(i == 0), stop=(i == n - 1))

            # ---- jump loop, software-pipelined ----
            # upv[:, t, 0, :] = c0*u_t (E0 path), [:, t, 1, :] = c1*u_t (E1).
            # E block for (t, kind): blk = kind*2 + t.
            # Matmuls group by DEST half mt into separate PSUM tiles; dest
            # half mt feeds source half t=mt next jump, so the DVE op for
            # upv[t=0] of jump j+1 runs as soon as jump j's mt=0 group (4
            # matmuls) stops, overlapping the mt=1 group -- the PE stream
            # stays gapless and the DVE ops hide under it.
            sacc_prev = saccp.tile([128, 2, 2, BH], f16, tag="sacc", name="sacc_init")
            nc.gpsimd.memset(sacc_prev[:], 0.0)
            pq = None
            upv = None
            for j in range(J):
                upv = state.tile([128, 2, 2, BH], f16, tag="upv", name=f"upv{j}")
                for t in range(2):
                    s = z[:, t, None, :] if j == 0 else pq[t][:, None, :]
                    nc.vector.tensor_tensor(
                        out=upv[:, t, :, :], in0=c01[:, t, :, :],
                        in1=s.broadcast_to([128, 2, BH]), op=mult)
                if j > 0:
                    warm(8 if j == J - 1 else 4, f"pw{j}")
                if j < J - 1:
                    pq = [psum.tile([128, BH], f32, tag=f"pq{mt}", name=f"pq{mt}_{j}")
                          for mt in range(2)]
                    for mt in range(2):
                        ms = slice(mt * 128, (mt + 1) * 128)
                        for i, (t, kind) in enumerate(
                                ((0, 0), (0, 1), (1, 0), (1, 1))):
                            nc.tensor.matmul(
                                pq[mt][:], E[:, kind * 2 + t, ms],
                                upv[:, t, kind, :],
                                start=(i == 0), stop=(i == 3),
                            )
                # running state sum for the final leaf projection: GPSIMD for
                # jumps 0..6; the final add runs on the (now idle) DVE so the
                # leaf matmuls aren't stuck behind the slower GPSIMD
                if j < J - 1:
                    sacc = saccp.tile([128, 2, 2, BH], f16, tag="sacc", name=f"sacc{j}")
                    nc.gpsimd.tensor_tensor(out=sacc[:], in0=sacc_prev[:],
                                            in1=upv[:], op=add)
                    sacc_prev = sacc
            sfin = state.tile([128, 2, 2, BH], f16, tag="sfin", name="sfin")
            nc.vector.tensor_tensor(out=sfin[:], in0=sacc_prev[:], in1=upv[:], op=add)

            # ---- leaf projection, once on the total state sum ----
            pleaf = psum_leaf.tile([BH, 256], f32, tag="pl", name="pl")
            for i, (t, kind) in enumerate(((0, 0), (1, 0), (0, 1), (1, 1))):
                nc.tensor.matmul(
                    pleaf[:], sfin[:, t, kind, :],
                    E[:, kind * 2 + t, 256:512],
                    start=(i == 0), stop=(i == 3),
                )

            # ---- output: undo the 2^15 state scale in the copy; DMA split
            # across both HWDGE queues for parallel descriptor dispatch ----
            o = work.tile([BH, 256], f32, tag="o", name="o")
            nc.vector.tensor_scalar_mul(o[:], pleaf[:], float(1.0 / SCALE))
            nc.sync.dma_start(p_out[:, 0:128], o[:, 0:128])
            nc.scalar.dma_start(p_out[:, 128:256], o[:, 128:256])

    nc.finalize()
    return nc


def _get_program():
    if "nc" not in _CACHE:
        _CACHE["nc"] = _build_program()
    return _CACHE["nc"]


def _prep_inputs(x, W, bias, M_left, M_right):
    """Host-side shard + layout prep. Core c -> graph c//2, batch half c%2."""
    in_maps = []
    mm_g, wt_g, bias_g = [], [], []
    for g in range(G):
        mm = np.zeros((4, 128, 512), np.float32)
        for side, M in ((0, M_left), (1, M_right)):
            mt = np.zeros((256, 512), np.float32)
            t = M[g].T  # (255, 511)
            mt[0:255, 0:255] = t[:, 0:255]
            mt[0:255, 256:512] = t[:, 255:511]
            mt[0:255, 255] = NEG
            mm[side * 2 + 0] = mt[0:128]
            mm[side * 2 + 1] = mt[128:256]
        mm_g.append(np.ascontiguousarray(
            mm.transpose(1, 0, 2).reshape(128, 2048)).astype(np.float16))
        wt = np.zeros((512, 256), np.float32)
        wt[:, 0:255] = W[g].T
        wt_g.append(wt)
        bp = np.zeros((128, 2), np.float32)
        bp[:, 0] = -bias[g][0:128]
        bp[0:127, 1] = -bias[g][128:255]
        bias_g.append(bp)
    xt_h = [np.ascontiguousarray(x[h * BH:(h + 1) * BH].T) for h in range(2)]
    wx_gh = {}
    for g in range(G):
        for h in range(2):
            wxf = np.zeros((128, 1282), np.float32)
            for k in range(4):
                wxf[:, k * 320:k * 320 + 256] = wt_g[g][k * 128:(k + 1) * 128]
                wxf[:, k * 320 + 256:k * 320 + 320] = xt_h[h][k * 128:(k + 1) * 128]
            wxf[:, 1280:1282] = bias_g[g]
            wx_gh[(g, h)] = np.ascontiguousarray(wxf).astype(np.float16)
    for c in range(NCORES):
        g, h = c // 2, c % 2
        in_maps.append({"mm": mm_g[g], "wx": wx_gh[(g, h)]})
    return in_maps


def _assemble(results):
    eps = np.float32(1e-5)
    ret = np.empty((B, L, G), np.float32)
    for c in range(NCORES):
        g, h = c // 2, c % 2
        ret[h * BH:(h + 1) * BH, :, g] = results[c]["out"]
    ret = np.where(ret > 0.0, ret, eps)
    ret = np.where(ret < 1.0, ret, np.float32(1.0) - eps)
    return ret.astype(np.float32)


def run_on_device(in_maps, trace=False, **kw):
    from concourse.bass_utils import run_bass_kernel_spmd
    nc = _get_program()
    return run_bass_kernel_spmd(nc, in_maps, list(range(NCORES)), trace=trace, **kw)


def kernel(x, W, bias, M_left, M_right):
    in_maps = _prep_inputs(
        np.asarray(x, np.float32), np.asarray(W, np.float32),
        np.asarray(bias, np.float32), np.asarray(M_left, np.float32),
        np.asarray(M_right, np.float32),
    )
    res = run_on_device(in_maps)
    return _assemble(res.results)
